# revision 1
# baseline (speedup 1.0000x reference)
"""Multi-head attention forward, distributed over 8 TRN2 NeuronCores.

Problem: x[2,2048,1024] -> QKV proj (16 heads x 64) -> softmax attention
-> output proj + bias -> [2,2048,1024], f32 I/O, bf16 tensor-engine compute.

Sharding: rows = flattened (batch, seq) = 4096 rows; core c owns rows
[c*512, (c+1)*512) -- cores 0-3 hold batch 0, cores 4-7 batch 1. Each core
projects Q/K/V for its own rows, all-gathers K^T and V (bf16) within its
4-core batch group in TWO chunks (so attention on chunk-0 keys overlaps the
chunk-1 AllGather), then computes attention for all 16 heads restricted to
its 512 query rows, and the output projection for those rows.

Host-side prep: x is transposed (x^T [D, rows]) and cast to bf16 along with
all weights before being fed to the device -- the kernel would cast to bf16
for the TensorEngine anyway, so this halves input DMA bytes and removes the
on-chip transpose/cast stages entirely.

Layouts (chosen so no transposes appear in the attention loop and every
matmul contracts over K=128 -- K=64 matmuls stream at half rate on TRN2):
  Q^T/K^T[hd, rows]   = W^T x^T  (lhsT = W natural); head PAIRS share a
                       128-partition tile (head 2p on rows 0-63, 2p+1 on
                       64-127).
  qTe/qTo[128, rows]  Q^T of the even/odd head of the pair with the OTHER
                       head's rows zeroed: scores lhsT is then the full
                       [128, 128] K-pair tile and the zero rows kill the
                       wrong-head contribution => full-rate K=128 matmul.
  S^T    [keys, q]    keys on partitions: the softmax reduction over keys is
                       done by the attention matmul itself -- V is augmented
                       with a ones column, making row 64 of att^T the
                       softmax denominator.
  att^T  [hd, q]      = (V_aug).T @ P^T, accumulated per AG-chunk in PSUM,
                       summed into SBUF bf16 accumulators (releases PSUM so
                       head-pairs pipeline across chunk arrivals).
  out    [rows, D]    = lhsT(att^T).T @ Wo natural (+ ones-row x bo matmul)
exp has no max subtraction (scores are ~N(0,1) after the 1/sqrt(64) scale
that is folded into the ACT activation scale).
"""

import ml_dtypes
import numpy as np

import concourse.bass as bass
import concourse.mybir as mybir
import concourse.tile as tile
from concourse import bacc
from concourse.bass_utils import run_bass_kernel_spmd

BF = mybir.dt.bfloat16
F32 = mybir.dt.float32
P = 128

N_CORES = 8
GROUP = 4   # cores per batch group (one AllGather group)
NCH = 2     # AllGather chunks (pipelined)


class Cfg:
    def __init__(self, rpc, d, n_heads, head_dim):
        self.RPC = rpc            # query rows per core
        self.D = d                # model dim
        self.H = n_heads
        self.HD = head_dim
        assert n_heads * head_dim == d
        self.NT_D = d // P        # dim tiles (= head pairs)
        self.NT_R = rpc // P      # row tiles
        self.KEYS = rpc * GROUP   # keys per batch group
        self.NT_K = self.KEYS // P
        self.HPT = P // head_dim  # heads per dim-tile
        assert self.HPT == 2
        assert self.NT_R % NCH == 0
        # AG chunk geometry (bounce buffers are [rows, RPC])
        self.KPC = rpc // NCH              # local keys per chunk
        self.KROWS_H = d * self.KPC // rpc   # K^T rows per chunk buffer
        self.VROWS_T = P * d // rpc          # rows per V row-tile
        self.RT_PER_CH = self.NT_R // NCH    # row tiles per chunk
        self.CH_ROWS = self.KROWS_H + self.RT_PER_CH * self.VROWS_T
        self.KTW = P * self.KPC // rpc       # chunk rows per K^T hd-tile


FULL = Cfg(rpc=512, d=1024, n_heads=16, head_dim=64)
SMALL = Cfg(rpc=256, d=256, n_heads=4, head_dim=64)


def _body(tc, nc, cfg, xT_in, wq_in, wk_in, wv_in, wo_in, bo_in, out_ext):
    c = cfg
    AF = mybir.ActivationFunctionType
    rg = [list(range(GROUP)), list(range(GROUP, 2 * GROUP))]
    HD1 = c.HD + 1
    chunk = min(c.D, 512)
    n_chunks = c.D // chunk
    from contextlib import ExitStack

    stack = ExitStack()
    dram = stack.enter_context(tc.tile_pool(name="dram", bufs=1, space="DRAM"))
    const = stack.enter_context(tc.tile_pool(name="const", bufs=1))
    persist = stack.enter_context(tc.tile_pool(name="persist", bufs=1))

    kv_in = [dram.tile([c.CH_ROWS, c.RPC], BF, name=f"kv_in{h}") for h in range(NCH)]
    kv_g = [
        dram.tile([GROUP * c.CH_ROWS, c.RPC], BF, name=f"kv_g{h}") for h in range(NCH)
    ]

    ones_row = const.tile([1, P], BF, tag="ones_row", name="ones_row")
    nc.vector.memset(ones_row[:], 1.0)
    bo_sb = const.tile([1, c.D], BF, tag="bo", name="bo_sb")
    nc.sync.dma_start(bo_sb[:], bo_in[:, :])

    def ptiles(shape, dt_, pfx, n, pool=None):
        pool = pool or persist
        return [pool.tile(shape, dt_, tag=f"{pfx}{t}", name=f"{pfx}{t}") for t in range(n)]

    xT = ptiles([P, c.RPC], BF, "xT", c.NT_D)
    qTe = ptiles([P, c.RPC], BF, "qTe", c.NT_D)
    qTo = ptiles([P, c.RPC], BF, "qTo", c.NT_D)
    attT = ptiles([P, c.RPC], BF, "attT", c.NT_D)
    # per-chunk K^T tiles so chunk-0 attention doesn't wait on chunk-1's AG
    kt_sb = [ptiles([P, c.KEYS // NCH], BF, f"kt{h}_", c.NT_D) for h in range(NCH)]
    v_aug = ptiles([P, c.H * HD1], BF, "va", c.NT_K)
    wo_sb = ptiles([P, c.D], BF, "wo", c.NT_D)

    with (
        tc.tile_pool(name="stage", bufs=2) as stage,
        tc.tile_pool(name="wpool", bufs=1) as wpool,
        tc.tile_pool(name="proj_psum", bufs=4, space="PSUM") as proj_psum,
    ):
        wq_sb = ptiles([P, c.D], BF, "wq", c.NT_D, pool=wpool)
        wk_sb = ptiles([P, c.D], BF, "wk", c.NT_D, pool=wpool)
        wv_sb = ptiles([P, c.D], BF, "wv", c.NT_D, pool=wpool)
        # ---- phase 0: load x^T and weights (already bf16, pre-transposed) ----
        for t in range(c.NT_D):
            nc.sync.dma_start(xT[t][:], xT_in[t * P : (t + 1) * P, :])
        for t in range(c.NT_D):
            nc.sync.dma_start(wk_sb[t][:], wk_in[t * P : (t + 1) * P, :])
        for t in range(c.NT_D):
            nc.sync.dma_start(wv_sb[t][:], wv_in[t * P : (t + 1) * P, :])
        for t in range(c.NT_D):
            nc.sync.dma_start(wq_sb[t][:], wq_in[t * P : (t + 1) * P, :])
        for t in range(c.NT_D):
            nc.sync.dma_start(wo_sb[t][:], wo_in[t * P : (t + 1) * P, :])

        # ---- phase 1: K^T, then V, feeding chunked bounce buffers ----
        kt_loc = []
        for m in range(c.NT_D):
            ps = proj_psum.tile([P, c.RPC], F32, tag="proj", name="proj_ps")
            for k in range(c.NT_D):
                nc.tensor.matmul(
                    ps[:],
                    wk_sb[k][:, m * P : (m + 1) * P],
                    xT[k][:],
                    start=(k == 0),
                    stop=(k == c.NT_D - 1),
                )
            kl = stage.tile([P, c.RPC], BF, tag=f"ktloc{m}", name=f"ktloc{m}", bufs=1)
            nc.vector.tensor_copy(kl[:], ps[:])
            kt_loc.append(kl)

        def emit_v_tile(rt):
            vloc = stage.tile([P, c.D], BF, tag="vloc", name="vloc", bufs=2)
            for n in range(n_chunks):
                ps = proj_psum.tile([P, chunk], F32, tag="proj", name="proj_ps")
                for k in range(c.NT_D):
                    nc.tensor.matmul(
                        ps[:],
                        xT[k][:, rt * P : (rt + 1) * P],
                        wv_sb[k][:, n * chunk : (n + 1) * chunk],
                        start=(k == 0),
                        stop=(k == c.NT_D - 1),
                    )
                nc.vector.tensor_copy(vloc[:, n * chunk : (n + 1) * chunk], ps[:])
            return vloc

        def emit_ag(h):
            nc.gpsimd.collective_compute(
                "AllGather",
                mybir.AluOpType.bypass,
                replica_groups=rg,
                ins=[kv_in[h][:].opt()],
                outs=[kv_g[h][:].opt()],
            )

        for h in range(NCH):
            for m in range(c.NT_D):
                nc.sync.dma_start(
                    kv_in[h][m * c.KTW : (m + 1) * c.KTW, :],
                    kt_loc[m][:, h * c.KPC : (h + 1) * c.KPC],
                )
            for lh in range(c.RT_PER_CH):
                rt = h * c.RT_PER_CH + lh
                vloc = emit_v_tile(rt)
                nc.sync.dma_start(
                    kv_in[h][
                        c.KROWS_H + lh * c.VROWS_T : c.KROWS_H + (lh + 1) * c.VROWS_T, :
                    ],
                    vloc[:],
                )
            if h == 0:
                # AG for chunk h>0 is emitted after chunk h-1's unpack DMAs so
                # the unpack gets the DMA queues before the next AG's wire
                # traffic floods them.
                emit_ag(0)

        # ---- Q^T with zero-padded even/odd variants (overlaps collectives) ----
        for m in range(c.NT_D):
            ps = proj_psum.tile([P, c.RPC], F32, tag="proj", name="proj_ps")
            for k in range(c.NT_D):
                nc.tensor.matmul(
                    ps[:],
                    wq_sb[k][:, m * P : (m + 1) * P],
                    xT[k][:],
                    start=(k == 0),
                    stop=(k == c.NT_D - 1),
                )
            nc.vector.tensor_copy(qTe[m][0 : c.HD, :], ps[0 : c.HD, :])
            nc.vector.memset(qTe[m][c.HD : P, :], 0.0)
            nc.vector.memset(qTo[m][0 : c.HD, :], 0.0)
            nc.vector.tensor_copy(qTo[m][c.HD : P, :], ps[c.HD : P, :])


    # ---- phases 3+4: per chunk: unpack (rank-interleaved), then attention ----
    def chunk_kts(h):
        # (global key-tile j, column block in the chunk-h kt tile)
        return [
            (r * c.NT_R + h * c.RT_PER_CH + lh, r * c.KPC + lh * P)
            for r in range(GROUP)
            for lh in range(c.RT_PER_CH)
        ]

    with (
        tc.tile_pool(name="vstage", bufs=8) as vstage,
        tc.tile_pool(name="accp", bufs=1) as accp,
        tc.tile_pool(name="pT", bufs=6) as pT_pool,
        tc.tile_pool(name="small", bufs=4) as small,
        tc.tile_pool(name="sc_psum", bufs=3, space="PSUM") as sc_psum,
        tc.tile_pool(name="att_psum", bufs=2, space="PSUM") as att_psum,
    ):
        acc_e = [accp.tile([HD1, c.RPC], BF, tag=f"acce{t}", name=f"acce{t}") for t in range(c.NT_D)]
        acc_o = [accp.tile([HD1, c.RPC], BF, tag=f"acco{t}", name=f"acco{t}") for t in range(c.NT_D)]

        for h in range(NCH):
            # unpack rank-major so early key-tiles become usable before the
            # whole gathered chunk lands (the next chunk's AG is still
            # occupying DMA bandwidth).
            for r in range(GROUP):
                for t in range(c.NT_D):
                    nc.sync.dma_start(
                        kt_sb[h][t][:, r * c.KPC : (r + 1) * c.KPC],
                        kv_g[h][
                            r * c.CH_ROWS + t * c.KTW : r * c.CH_ROWS + (t + 1) * c.KTW,
                            :,
                        ],
                    )
                for lh in range(c.RT_PER_CH):
                    j = r * c.NT_R + h * c.RT_PER_CH + lh
                    vst = vstage.tile([P, c.D], BF, tag="vst", name="vst")
                    nc.sync.dma_start(
                        vst[:],
                        kv_g[h][
                            r * c.CH_ROWS
                            + c.KROWS_H
                            + lh * c.VROWS_T : r * c.CH_ROWS
                            + c.KROWS_H
                            + (lh + 1) * c.VROWS_T,
                            :,
                        ],
                    )
                    nc.vector.tensor_copy(
                        v_aug[j][:].rearrange("p (h e) -> p h e", e=HD1)[:, :, 0 : c.HD],
                        vst[:].rearrange("p (h e) -> p h e", e=c.HD),
                    )
                    ones_col = v_aug[j][:].rearrange("p (h e) -> p h e", e=HD1)[
                        :, :, c.HD : HD1
                    ]
                    nc.vector.memset(ones_col, 1.0)

            if h + 1 < NCH:
                emit_ag(h + 1)

            for p in range(c.NT_D):
                he, ho = 2 * p, 2 * p + 1
                att_e = att_psum.tile([HD1, c.RPC], F32, tag="att", name="att_e")
                att_o = att_psum.tile([HD1, c.RPC], F32, tag="att", name="att_o")
                kts = chunk_kts(h)
                for idx, (j, col) in enumerate(kts):
                    sc = sc_psum.tile([P, 2 * c.RPC], F32, tag="scores", name="sc_ps")
                    nc.tensor.matmul(
                        sc[:, 0 : c.RPC],
                        kt_sb[h][p][:, col : col + P],
                        qTe[p][:],
                        start=True,
                        stop=True,
                    )
                    nc.tensor.matmul(
                        sc[:, c.RPC : 2 * c.RPC],
                        kt_sb[h][p][:, col : col + P],
                        qTo[p][:],
                        start=True,
                        stop=True,
                    )
                    pT = pT_pool.tile([P, 2 * c.RPC], BF, tag="pT", name="pT")
                    nc.scalar.activation(
                        pT[:], sc[:], AF.Exp, scale=1.0 / float(np.sqrt(c.HD))
                    )
                    nc.tensor.matmul(
                        att_e[:],
                        v_aug[j][:, he * HD1 : (he + 1) * HD1],
                        pT[:, 0 : c.RPC],
                        start=(idx == 0),
                        stop=(idx == len(kts) - 1),
                    )
                    nc.tensor.matmul(
                        att_o[:],
                        v_aug[j][:, ho * HD1 : (ho + 1) * HD1],
                        pT[:, c.RPC : 2 * c.RPC],
                        start=(idx == 0),
                        stop=(idx == len(kts) - 1),
                    )
                if h == 0:
                    nc.vector.tensor_copy(acc_e[p][:], att_e[:])
                    nc.vector.tensor_copy(acc_o[p][:], att_o[:])
                else:
                    nc.vector.tensor_add(acc_e[p][:], att_e[:], acc_e[p][:])
                    nc.vector.tensor_add(acc_o[p][:], att_o[:], acc_o[p][:])

                if h == NCH - 1:
                    # normalization: denominators live in row HD of the accs
                    den_e = small.tile([1, c.RPC], F32, tag="dene", name="dene", bufs=1)
                    nc.vector.tensor_copy(den_e[:], acc_e[p][c.HD : HD1, :])
                    den_o = small.tile([1, c.RPC], F32, tag="deno", name="deno", bufs=1)
                    nc.vector.tensor_copy(den_o[:], acc_o[p][c.HD : HD1, :])
                    rcp_e = small.tile([1, c.RPC], F32, tag="rcpe", name="rcpe", bufs=1)
                    nc.vector.reciprocal_approx_fast(rcp_e[:], den_e[:])
                    rcp_o = small.tile([1, c.RPC], F32, tag="rcpo", name="rcpo", bufs=1)
                    nc.vector.reciprocal_approx_fast(rcp_o[:], den_o[:])
                    rcpb_e = small.tile([c.HD, c.RPC], F32, tag="rcpbe", name="rcpbe", bufs=1)
                    nc.gpsimd.partition_broadcast(rcpb_e[:], rcp_e[:])
                    rcpb_o = small.tile([c.HD, c.RPC], F32, tag="rcpbo", name="rcpbo", bufs=1)
                    nc.gpsimd.partition_broadcast(rcpb_o[:], rcp_o[:])
                    nc.vector.tensor_mul(
                        attT[p][0 : c.HD, :], acc_e[p][0 : c.HD, :], rcpb_e[:]
                    )
                    nc.vector.tensor_mul(
                        attT[p][c.HD : P, :], acc_o[p][0 : c.HD, :], rcpb_o[:]
                    )

        # ---- phase 5: output projection + bias ----
        for rt in range(c.NT_R):
            out_sb = small.tile([P, c.D], F32, tag="outsb", name="outsb", bufs=2)
            for n in range(n_chunks):
                po = sc_psum.tile([P, chunk], F32, tag="scores", name="sc_ps")
                for k in range(c.NT_D):
                    nc.tensor.matmul(
                        po[:],
                        attT[k][:, rt * P : (rt + 1) * P],
                        wo_sb[k][:, n * chunk : (n + 1) * chunk],
                        start=(k == 0),
                        stop=False,
                    )
                nc.tensor.matmul(
                    po[:],
                    ones_row[:],
                    bo_sb[:, n * chunk : (n + 1) * chunk],
                    start=False,
                    stop=True,
                )
                nc.vector.tensor_copy(out_sb[:, n * chunk : (n + 1) * chunk], po[:])
            nc.sync.dma_start(out_ext[rt * P : (rt + 1) * P, :], out_sb[:])

    stack.close()


def build_nc(cfg):
    nc = bacc.Bacc(
        "TRN2", target_bir_lowering=False, debug=False, num_devices=N_CORES
    )
    c = cfg
    xT_in = nc.dram_tensor("xT", [c.D, c.RPC], BF, kind="ExternalInput")
    wq_in = nc.dram_tensor("Wq", [c.D, c.D], BF, kind="ExternalInput")
    wk_in = nc.dram_tensor("Wk", [c.D, c.D], BF, kind="ExternalInput")
    wv_in = nc.dram_tensor("Wv", [c.D, c.D], BF, kind="ExternalInput")
    wo_in = nc.dram_tensor("Wo", [c.D, c.D], BF, kind="ExternalInput")
    bo_in = nc.dram_tensor("bo", [1, c.D], BF, kind="ExternalInput")
    out_ext = nc.dram_tensor("out", [c.RPC, c.D], F32, kind="ExternalOutput")

    with tile.TileContext(nc) as tc:
        _body(
            tc, nc, cfg,
            xT_in.ap(), wq_in.ap(), wk_in.ap(), wv_in.ap(), wo_in.ap(),
            bo_in.ap(), out_ext.ap(),
        )
    nc.compile()
    return nc


_cached_nc = None


def _bf16(a):
    return np.ascontiguousarray(np.asarray(a, dtype=np.float32)).astype(
        ml_dtypes.bfloat16
    )


def prep_in_maps(c, x, Wq, Wk, Wv, Wo, bo):
    xf = np.ascontiguousarray(np.asarray(x, dtype=np.float32)).reshape(-1, c.D)
    wq, wk, wv, wo = _bf16(Wq), _bf16(Wk), _bf16(Wv), _bf16(Wo)
    bob = _bf16(bo).reshape(1, c.D)
    return [
        {
            "xT": np.ascontiguousarray(
                xf[cid * c.RPC : (cid + 1) * c.RPC].T.astype(ml_dtypes.bfloat16)
            ),
            "Wq": wq, "Wk": wk, "Wv": wv, "Wo": wo, "bo": bob,
        }
        for cid in range(N_CORES)
    ]


def kernel(x, Wq, Wk, Wv, Wo, bo):
    global _cached_nc
    c = FULL
    if _cached_nc is None:
        _cached_nc = build_nc(c)
    nc = _cached_nc

    in_maps = prep_in_maps(c, x, Wq, Wk, Wv, Wo, bo)
    res = run_bass_kernel_spmd(nc, in_maps, list(range(N_CORES)))
    out = np.concatenate([res.results[cid]["out"] for cid in range(N_CORES)], axis=0)
    return out.reshape(np.asarray(x).shape).astype(np.float32)

